# revision 5
# baseline (speedup 1.0000x reference)
"""Trainium2 Bass kernel for nn_MultiHeadAttention (B=2, S=2048, D=1024, H=16).

Sharding (8 cores): data-parallel over batch (2) x tensor-parallel over
head groups (4 groups of 4 heads). Core c handles batch c//4, heads
4*(c%4) .. 4*(c%4)+3.  Each core computes the full attention for its
heads plus its slice of the output projection; the host sums the 4
partial output projections per batch and adds bo.

On-chip layouts (per core):
  qT, kT  [256 feat, 2048 seq]   (features on partitions)
  v       [2048 keys, 4*65]      (per head: 64 feats + ones column)
  scoresT [keys, queries] tiles -> exp (scalar engine, scale=1/8 fused,
          max-subtract skipped: softmax is shift invariant and scores
          are O(1) here, fp32 exp is exact enough)
  ctxT    [65, queries] accumulated over key tiles; row 64 = sum of exp
          (from the ones column) -> reciprocal -> broadcast -> scale.
Matmuls run as float32r (full-rate fp32 PE mode, free dim 512).
"""

import sys

for _p in ("/opt/trn_rl_repo",):
    if _p not in sys.path:
        sys.path.insert(0, _p)

from contextlib import ExitStack

import numpy as np

import concourse.bass as bass
import concourse.tile as tile
from concourse import bacc, mybir
from concourse.bass_utils import run_bass_kernel_spmd

B, S, D, H = 2, 2048, 1024, 16
HD = D // H            # 64 head dim
NG = 4                 # head groups (cores per batch)
NHC = H // NG          # 4 heads per core
FS = NHC * HD          # 256 features per core
P = 128
DK = D // P            # 8 contraction tiles for projections
SK = S // P            # 16 key tiles
NQ = S // 512          # 4 query chunks
FK = FS // P           # 2 feature tiles for qT/kT/ctxT

f32 = mybir.dt.float32
f32r = mybir.dt.float32r
EXP = mybir.ActivationFunctionType.Exp


def _emit(ctx: ExitStack, tc, nc, io):
    QT, KT, VT, WqT, WkT, WvT, WoT, bq, bk, bv, OUTP = io

    xt = ctx.enter_context(tc.tile_pool(name="xt", bufs=20))
    wp = ctx.enter_context(tc.tile_pool(name="wp", bufs=1))
    per = ctx.enter_context(tc.tile_pool(name="per", bufs=1))
    exp = ctx.enter_context(tc.tile_pool(name="exp", bufs=3))
    nrm = ctx.enter_context(tc.tile_pool(name="nrm", bufs=3))
    ctxp = ctx.enter_context(tc.tile_pool(name="ctxp", bufs=2))
    outp = ctx.enter_context(tc.tile_pool(name="outp", bufs=4))
    sc_ps = ctx.enter_context(tc.tile_pool(name="sc_ps", bufs=2, space="PSUM"))
    acc_ps = ctx.enter_context(tc.tile_pool(name="acc_ps", bufs=4, space="PSUM"))

    # ---- weights / biases (persistent) ----
    wq = [wp.tile([P, FS], f32r, tag=f"wq{k}", name=f"wq{k}") for k in range(DK)]
    wk = [wp.tile([P, FS], f32r, tag=f"wk{k}", name=f"wk{k}") for k in range(DK)]
    wv = [wp.tile([P, FS], f32r, tag=f"wv{k}", name=f"wv{k}") for k in range(DK)]
    wo = [wp.tile([P, D], f32r, tag=f"wo{f}", name=f"wo{f}") for f in range(FK)]
    for k in range(DK):
        nc.sync.dma_start(wq[k][:], WqT[k * P:(k + 1) * P, :])
        nc.sync.dma_start(wk[k][:], WkT[k * P:(k + 1) * P, :])
        nc.sync.dma_start(wv[k][:], WvT[k * P:(k + 1) * P, :])
    for f in range(FK):
        nc.sync.dma_start(wo[f][:], WoT[f * P:(f + 1) * P, :])
    bq_t = [wp.tile([P, 1], f32, tag=f"bq{f}", name=f"bqt{f}") for f in range(FK)]
    bk_t = [wp.tile([P, 1], f32, tag=f"bk{f}", name=f"bkt{f}") for f in range(FK)]
    for f in range(FK):
        nc.sync.dma_start(bq_t[f][:], bq[f * P:(f + 1) * P, :])
        nc.sync.dma_start(bk_t[f][:], bk[f * P:(f + 1) * P, :])
    bv_t = wp.tile([P, FS], f32, tag="bv")
    nc.sync.dma_start(bv_t[:], bv.to_broadcast((P, FS)))
    ones_t = wp.tile([P, NHC], f32, tag="ones")
    nc.vector.memset(ones_t[:], 1.0)

    # ---- persistent activations ----
    kT = [per.tile([P, S], f32r, tag=f"kT{f}", name=f"kTs{f}") for f in range(FK)]
    qT = [per.tile([P, S], f32r, tag=f"qT{f}", name=f"qTs{f}") for f in range(FK)]
    vsb = [per.tile([P, NHC * (HD + 1)], f32r, tag=f"v{t}", name=f"vs{t}") for t in range(SK)]

    # ---- input streaming tiles: [128, 1024] halves ----
    def load_xt(src):
        tiles = {}
        for k in range(DK):
            for hf in range(2):
                t = xt.tile([P, 1024], f32r, tag="xt", name="xtile")
                nc.sync.dma_start(t[:], src[k * P:(k + 1) * P,
                                            hf * 1024:(hf + 1) * 1024])
                tiles[(k, hf)] = t
        return tiles

    def proj_T(src_tiles, w, b_t, dst):
        # dst[f][:, nc512] = (W_slice @ X^T + b), features on partitions
        for ncol in range(NQ):
            hf, off = divmod(ncol * 512, 1024)
            for f in range(FK):
                ps = acc_ps.tile([P, 512], f32, tag="acc")
                for k in range(DK):
                    nc.tensor.matmul(
                        ps[:],
                        w[k][:, f * P:(f + 1) * P],
                        src_tiles[(k, hf)][:, off:off + 512],
                        start=(k == 0), stop=(k == DK - 1),
                    )
                nc.vector.tensor_scalar_add(
                    dst[f][:, ncol * 512:(ncol + 1) * 512], ps[:], b_t[f][:])

    # K projection first (attention needs all of kT), then Q, then V.
    kt_tiles = load_xt(KT)
    proj_T(kt_tiles, wk, bk_t, kT)
    qt_tiles = load_xt(QT)
    proj_T(qt_tiles, wq, bq_t, qT)

    vt_tiles = load_xt(VT)
    for t in range(SK):
        hf, off = divmod(t * P, 1024)
        ps = acc_ps.tile([P, FS], f32, tag="acc")
        for k in range(DK):
            nc.tensor.matmul(
                ps[:],
                vt_tiles[(k, hf)][:, off:off + P],
                wv[k][:],
                start=(k == 0), stop=(k == DK - 1),
            )
        for h in range(NHC):
            nc.vector.tensor_add(
                vsb[t][:, h * (HD + 1):h * (HD + 1) + HD],
                ps[:, h * HD:(h + 1) * HD],
                bv_t[:, h * HD:(h + 1) * HD])
        nc.vector.tensor_copy(vsb[t][:, HD:NHC * (HD + 1):HD + 1], ones_t[:])

    # ---- attention + output projection ----
    CH = 2  # key tiles per exp chunk
    for j in range(NQ):
        ctxn = [ctxp.tile([P, 512], f32r, tag=f"ctxn{f}", name=f"ctxn{f}") for f in range(FK)]
        for h in range(NHC):
            fq, rq = divmod(h * HD, P)
            qv = qT[fq][rq:rq + HD, j * 512:(j + 1) * 512]
            ctx_ps = acc_ps.tile([HD + 1, 512], f32, tag="acc")
            for c in range(SK // CH):
                sc = sc_ps.tile([P, CH * 512], f32, tag="sc")
                for t in range(CH):
                    kt2 = c * CH + t
                    nc.tensor.matmul(
                        sc[:, t * 512:(t + 1) * 512],
                        kT[fq][rq:rq + HD, kt2 * P:(kt2 + 1) * P],
                        qv,
                        start=True, stop=True,
                    )
                ex = exp.tile([P, CH * 512], f32r, tag="ex")
                nc.scalar.activation(ex[:], sc[:], EXP, scale=1.0 / (HD ** 0.5))
                for t in range(CH):
                    kt2 = c * CH + t
                    nc.tensor.matmul(
                        ctx_ps[:],
                        vsb[kt2][:, h * (HD + 1):(h + 1) * (HD + 1)],
                        ex[:, t * 512:(t + 1) * 512],
                        start=(kt2 == 0), stop=(kt2 == SK - 1),
                    )
            rc = nrm.tile([1, 512], f32, tag="rc")
            nc.vector.reciprocal(rc[:], ctx_ps[HD:HD + 1, :])
            rb = nrm.tile([HD, 512], f32, tag="rb")
            nc.gpsimd.partition_broadcast(rb[:], rc[:])
            nc.vector.tensor_mul(
                ctxn[fq][rq:rq + HD, :], ctx_ps[0:HD, :], rb[:])

        for mt in range(4):
            for oc in range(2):
                ps = acc_ps.tile([P, 512], f32, tag="acc")
                for f in range(FK):
                    nc.tensor.matmul(
                        ps[:],
                        ctxn[f][:, mt * P:(mt + 1) * P],
                        wo[f][:, oc * 512:(oc + 1) * 512],
                        start=(f == 0), stop=(f == FK - 1),
                    )
                ob = outp.tile([P, 512], f32, tag="ob")
                nc.vector.tensor_copy(ob[:], ps[:])
                nc.sync.dma_start(
                    OUTP[j * 512 + mt * P: j * 512 + (mt + 1) * P,
                         oc * 512:(oc + 1) * 512], ob[:])


_CACHE = {}


def _build():
    if "nc" in _CACHE:
        return _CACHE["nc"]
    nc = bacc.Bacc("TRN2", target_bir_lowering=False, debug=False)
    QT = nc.dram_tensor("QT", [D, S], f32r, kind="ExternalInput").ap()
    KT = nc.dram_tensor("KT", [D, S], f32r, kind="ExternalInput").ap()
    VT = nc.dram_tensor("VT", [D, S], f32r, kind="ExternalInput").ap()
    WqT = nc.dram_tensor("WqT", [D, FS], f32r, kind="ExternalInput").ap()
    WkT = nc.dram_tensor("WkT", [D, FS], f32r, kind="ExternalInput").ap()
    WvT = nc.dram_tensor("WvT", [D, FS], f32r, kind="ExternalInput").ap()
    WoT = nc.dram_tensor("WoT", [FS, D], f32r, kind="ExternalInput").ap()
    bq = nc.dram_tensor("bq", [FS, 1], f32, kind="ExternalInput").ap()
    bk = nc.dram_tensor("bk", [FS, 1], f32, kind="ExternalInput").ap()
    bv = nc.dram_tensor("bv", [1, FS], f32, kind="ExternalInput").ap()
    OUTP = nc.dram_tensor("OUTP", [S, D], f32, kind="ExternalOutput").ap()
    with tile.TileContext(nc) as tc, ExitStack() as ctx:
        _emit(ctx, tc, nc, (QT, KT, VT, WqT, WkT, WvT, WoT, bq, bk, bv, OUTP))
    nc.compile()
    _CACHE["nc"] = nc
    return nc


def _in_maps(Q, K, V, Wq, bq, Wk, bk, Wv, bv, Wo, bo):
    c = np.ascontiguousarray
    QTb = [c(Q[b].T) for b in range(B)]
    KTb = [c(K[b].T) for b in range(B)]
    VTb = [c(V[b].T) for b in range(B)]
    maps = []
    for core in range(8):
        b, g = divmod(core, NG)
        sl = slice(g * FS, (g + 1) * FS)
        maps.append({
            "QT": QTb[b], "KT": KTb[b], "VT": VTb[b],
            "WqT": c(Wq[sl, :].T), "WkT": c(Wk[sl, :].T),
            "WvT": c(Wv[sl, :].T), "WoT": c(Wo[:, sl].T),
            "bq": c(bq[sl].reshape(FS, 1)), "bk": c(bk[sl].reshape(FS, 1)),
            "bv": c(bv[sl].reshape(1, FS)),
        })
    return maps


def kernel(Q, K, V, Wq, bq, Wk, bk, Wv, bv, Wo, bo, _return_raw=False):
    nc = _build()
    maps = _in_maps(Q, K, V, Wq, bq, Wk, bk, Wv, bv, Wo, bo)
    res = run_bass_kernel_spmd(nc, maps, core_ids=list(range(8)))
    out = np.empty((B, S, D), np.float32)
    for b in range(B):
        acc = res.results[b * NG]["OUTP"].astype(np.float32)
        for g in range(1, NG):
            acc = acc + res.results[b * NG + g]["OUTP"]
        out[b] = acc + np.asarray(bo, np.float32)[None, :]
    return out


# revision 6
# speedup vs baseline: 1.0939x; 1.0939x over previous
"""Trainium2 Bass kernel for nn_MultiHeadAttention (B=2, S=2048, D=1024, H=16).

Sharding (8 cores): data-parallel over batch (2) x tensor-parallel over
head groups (4 groups of 4 heads). Core c handles batch c//4, heads
4*(c%4) .. 4*(c%4)+3.  Each core computes the full attention for its
heads plus its slice of the output projection; the host sums the 4
partial output projections per batch and adds bo.

On-chip layouts (per core):
  qT, kT  [256 feat, 2048 seq]   (features on partitions)
  v       [2048 keys, 4*65]      (per head: 64 feats + ones column)
  scoresT [keys, queries] tiles -> exp (scalar engine, scale=1/8 fused,
          max-subtract skipped: softmax is shift invariant and scores
          are O(1) here, fp32 exp is exact enough)
  ctxT    [65, queries] accumulated over key tiles; row 64 = sum of exp
          (from the ones column) -> reciprocal -> broadcast -> scale.
Matmuls run as float32r (full-rate fp32 PE mode, free dim 512).
"""

import sys

for _p in ("/opt/trn_rl_repo",):
    if _p not in sys.path:
        sys.path.insert(0, _p)

from contextlib import ExitStack

import ml_dtypes
import numpy as np

import concourse.bass as bass
import concourse.tile as tile
from concourse import bacc, mybir
from concourse.bass_utils import run_bass_kernel_spmd

B, S, D, H = 2, 2048, 1024, 16
HD = D // H            # 64 head dim
NG = 4                 # head groups (cores per batch)
NHC = H // NG          # 4 heads per core
FS = NHC * HD          # 256 features per core
P = 128
DK = D // P            # 8 contraction tiles for projections
SK = S // P            # 16 key tiles
NQ = S // 512          # 4 query chunks
FK = FS // P           # 2 feature tiles for qT/kT/ctxT

f32 = mybir.dt.float32
f32r = mybir.dt.float32r
bf16 = mybir.dt.bfloat16
EXP = mybir.ActivationFunctionType.Exp


def _emit(ctx: ExitStack, tc, nc, io):
    QT, KT, VT, WqT, WkT, WvT, WoT, bq, bk, bv, OUTP = io

    xt = ctx.enter_context(tc.tile_pool(name="xt", bufs=20))
    wp = ctx.enter_context(tc.tile_pool(name="wp", bufs=1))
    per = ctx.enter_context(tc.tile_pool(name="per", bufs=1))
    exp = ctx.enter_context(tc.tile_pool(name="exp", bufs=3))
    nrm = ctx.enter_context(tc.tile_pool(name="nrm", bufs=3))
    ctxp = ctx.enter_context(tc.tile_pool(name="ctxp", bufs=2))
    outp = ctx.enter_context(tc.tile_pool(name="outp", bufs=4))
    sc_ps = ctx.enter_context(tc.tile_pool(name="sc_ps", bufs=2, space="PSUM"))
    acc_ps = ctx.enter_context(tc.tile_pool(name="acc_ps", bufs=4, space="PSUM"))

    # ---- weights / biases (persistent) ----
    wq = [wp.tile([P, FS], f32r, tag=f"wq{k}", name=f"wq{k}") for k in range(DK)]
    wk = [wp.tile([P, FS], f32r, tag=f"wk{k}", name=f"wk{k}") for k in range(DK)]
    wv = [wp.tile([P, FS], f32r, tag=f"wv{k}", name=f"wv{k}") for k in range(DK)]
    wo = [wp.tile([P, D], bf16, tag=f"wo{f}", name=f"wo{f}") for f in range(FK)]
    for k in range(DK):
        nc.sync.dma_start(wq[k][:], WqT[k * P:(k + 1) * P, :])
        nc.sync.dma_start(wk[k][:], WkT[k * P:(k + 1) * P, :])
        nc.sync.dma_start(wv[k][:], WvT[k * P:(k + 1) * P, :])
    for f in range(FK):
        nc.sync.dma_start(wo[f][:], WoT[f * P:(f + 1) * P, :])
    bq_t = [wp.tile([P, 1], f32, tag=f"bq{f}", name=f"bqt{f}") for f in range(FK)]
    bk_t = [wp.tile([P, 1], f32, tag=f"bk{f}", name=f"bkt{f}") for f in range(FK)]
    for f in range(FK):
        nc.sync.dma_start(bq_t[f][:], bq[f * P:(f + 1) * P, :])
        nc.sync.dma_start(bk_t[f][:], bk[f * P:(f + 1) * P, :])
    bv_t = wp.tile([P, FS], f32, tag="bv")
    nc.sync.dma_start(bv_t[:], bv.to_broadcast((P, FS)))
    ones_t = wp.tile([P, NHC], f32, tag="ones")
    nc.vector.memset(ones_t[:], 1.0)

    # ---- persistent activations ----
    kT = [per.tile([P, S], bf16, tag=f"kT{f}", name=f"kTs{f}") for f in range(FK)]
    qT = [per.tile([P, S], bf16, tag=f"qT{f}", name=f"qTs{f}") for f in range(FK)]
    vsb = [per.tile([P, NHC * (HD + 1)], bf16, tag=f"v{t}", name=f"vs{t}") for t in range(SK)]

    # ---- input streaming tiles: [128, 1024] halves ----
    def load_xt(src):
        tiles = {}
        for k in range(DK):
            for hf in range(2):
                t = xt.tile([P, 1024], f32r, tag="xt", name="xtile")
                nc.sync.dma_start(t[:], src[k * P:(k + 1) * P,
                                            hf * 1024:(hf + 1) * 1024])
                tiles[(k, hf)] = t
        return tiles

    def proj_T(src_tiles, w, b_t, dst):
        # dst[f][:, nc512] = (W_slice @ X^T + b), features on partitions
        for ncol in range(NQ):
            hf, off = divmod(ncol * 512, 1024)
            for f in range(FK):
                ps = acc_ps.tile([P, 512], f32, tag="acc")
                for k in range(DK):
                    nc.tensor.matmul(
                        ps[:],
                        w[k][:, f * P:(f + 1) * P],
                        src_tiles[(k, hf)][:, off:off + 512],
                        start=(k == 0), stop=(k == DK - 1),
                    )
                nc.vector.tensor_scalar_add(
                    dst[f][:, ncol * 512:(ncol + 1) * 512], ps[:], b_t[f][:])

    # K projection first (attention needs all of kT), then Q, then V.
    kt_tiles = load_xt(KT)
    proj_T(kt_tiles, wk, bk_t, kT)
    qt_tiles = load_xt(QT)
    proj_T(qt_tiles, wq, bq_t, qT)

    vt_tiles = load_xt(VT)
    for t in range(SK):
        hf, off = divmod(t * P, 1024)
        ps = acc_ps.tile([P, FS], f32, tag="acc")
        for k in range(DK):
            nc.tensor.matmul(
                ps[:],
                vt_tiles[(k, hf)][:, off:off + P],
                wv[k][:],
                start=(k == 0), stop=(k == DK - 1),
            )
        for h in range(NHC):
            nc.vector.tensor_add(
                vsb[t][:, h * (HD + 1):h * (HD + 1) + HD],
                ps[:, h * HD:(h + 1) * HD],
                bv_t[:, h * HD:(h + 1) * HD])
        nc.vector.tensor_copy(vsb[t][:, HD:NHC * (HD + 1):HD + 1], ones_t[:])

    # ---- attention + output projection ----
    CH = 2  # key tiles per exp chunk
    for j in range(NQ):
        ctxn = [ctxp.tile([P, 512], bf16, tag=f"ctxn{f}", name=f"ctxn{f}") for f in range(FK)]
        for h in range(NHC):
            fq, rq = divmod(h * HD, P)
            qv = qT[fq][rq:rq + HD, j * 512:(j + 1) * 512]
            ctx_ps = acc_ps.tile([HD + 1, 512], f32, tag="acc")
            for c in range(SK // CH):
                sc = sc_ps.tile([P, CH * 512], f32, tag="sc")
                for t in range(CH):
                    kt2 = c * CH + t
                    nc.tensor.matmul(
                        sc[:, t * 512:(t + 1) * 512],
                        kT[fq][rq:rq + HD, kt2 * P:(kt2 + 1) * P],
                        qv,
                        start=True, stop=True,
                    )
                ex = exp.tile([P, CH * 512], bf16, tag="ex")
                nc.scalar.activation(ex[:], sc[:], EXP, scale=1.0 / (HD ** 0.5))
                for t in range(CH):
                    kt2 = c * CH + t
                    nc.tensor.matmul(
                        ctx_ps[:],
                        vsb[kt2][:, h * (HD + 1):(h + 1) * (HD + 1)],
                        ex[:, t * 512:(t + 1) * 512],
                        start=(kt2 == 0), stop=(kt2 == SK - 1),
                    )
            rc = nrm.tile([1, 512], f32, tag="rc")
            nc.vector.reciprocal(rc[:], ctx_ps[HD:HD + 1, :])
            rb = nrm.tile([HD, 512], f32, tag="rb")
            nc.gpsimd.partition_broadcast(rb[:], rc[:])
            nc.vector.tensor_mul(
                ctxn[fq][rq:rq + HD, :], ctx_ps[0:HD, :], rb[:])

        for mt in range(4):
            for oc in range(2):
                ps = acc_ps.tile([P, 512], f32, tag="acc")
                for f in range(FK):
                    nc.tensor.matmul(
                        ps[:],
                        ctxn[f][:, mt * P:(mt + 1) * P],
                        wo[f][:, oc * 512:(oc + 1) * 512],
                        start=(f == 0), stop=(f == FK - 1),
                    )
                ob = outp.tile([P, 512], f32, tag="ob")
                nc.vector.tensor_copy(ob[:], ps[:])
                nc.sync.dma_start(
                    OUTP[j * 512 + mt * P: j * 512 + (mt + 1) * P,
                         oc * 512:(oc + 1) * 512], ob[:])


_CACHE = {}


def _build():
    if "nc" in _CACHE:
        return _CACHE["nc"]
    nc = bacc.Bacc("TRN2", target_bir_lowering=False, debug=False)
    QT = nc.dram_tensor("QT", [D, S], f32r, kind="ExternalInput").ap()
    KT = nc.dram_tensor("KT", [D, S], f32r, kind="ExternalInput").ap()
    VT = nc.dram_tensor("VT", [D, S], f32r, kind="ExternalInput").ap()
    WqT = nc.dram_tensor("WqT", [D, FS], f32r, kind="ExternalInput").ap()
    WkT = nc.dram_tensor("WkT", [D, FS], f32r, kind="ExternalInput").ap()
    WvT = nc.dram_tensor("WvT", [D, FS], f32r, kind="ExternalInput").ap()
    WoT = nc.dram_tensor("WoT", [FS, D], bf16, kind="ExternalInput").ap()
    bq = nc.dram_tensor("bq", [FS, 1], f32, kind="ExternalInput").ap()
    bk = nc.dram_tensor("bk", [FS, 1], f32, kind="ExternalInput").ap()
    bv = nc.dram_tensor("bv", [1, FS], f32, kind="ExternalInput").ap()
    OUTP = nc.dram_tensor("OUTP", [S, D], f32, kind="ExternalOutput").ap()
    with tile.TileContext(nc) as tc, ExitStack() as ctx:
        _emit(ctx, tc, nc, (QT, KT, VT, WqT, WkT, WvT, WoT, bq, bk, bv, OUTP))
    nc.compile()
    _CACHE["nc"] = nc
    return nc


def _in_maps(Q, K, V, Wq, bq, Wk, bk, Wv, bv, Wo, bo):
    c = np.ascontiguousarray
    QTb = [c(Q[b].T) for b in range(B)]
    KTb = [c(K[b].T) for b in range(B)]
    VTb = [c(V[b].T) for b in range(B)]
    maps = []
    for core in range(8):
        b, g = divmod(core, NG)
        sl = slice(g * FS, (g + 1) * FS)
        maps.append({
            "QT": QTb[b], "KT": KTb[b], "VT": VTb[b],
            "WqT": c(Wq[sl, :].T), "WkT": c(Wk[sl, :].T),
            "WvT": c(Wv[sl, :].T),
            "WoT": c(Wo[:, sl].T).astype(ml_dtypes.bfloat16),
            "bq": c(bq[sl].reshape(FS, 1)), "bk": c(bk[sl].reshape(FS, 1)),
            "bv": c(bv[sl].reshape(1, FS)),
        })
    return maps


def kernel(Q, K, V, Wq, bq, Wk, bk, Wv, bv, Wo, bo, _return_raw=False):
    nc = _build()
    maps = _in_maps(Q, K, V, Wq, bq, Wk, bk, Wv, bv, Wo, bo)
    res = run_bass_kernel_spmd(nc, maps, core_ids=list(range(8)))
    out = np.empty((B, S, D), np.float32)
    for b in range(B):
        acc = res.results[b * NG]["OUTP"].astype(np.float32)
        for g in range(1, NG):
            acc = acc + res.results[b * NG + g]["OUTP"]
        out[b] = acc + np.asarray(bo, np.float32)[None, :]
    return out
